# revision 53
# baseline (speedup 1.0000x reference)
"""Trainium2 Bass kernel for a GPT-2 style transformer block (nn_Block_16690242913196).

Sharding (8 NeuronCores, identical SPMD program):
  - QKV/proj/MLP: token-parallel (core i owns 512 flat tokens).
  - Attention: head-parallel (core i owns heads {2i, 2i+1} for ALL tokens).
  - Collective 1: AllGather of rstd-scaled activations (+ mu*rstd row,
    ones row) in fp8. LN1 is folded into the QKV matmuls via rank-1
    correction rows appended to each PSUM chain.
  - Collectives 2a/2b: per-head AllToAll of attention outputs (+ the
    softmax-denominator row). Head 0's AllToAll fires at ~50% of the
    attention phase and hides under head 1's compute.

Perf structure vs the original baseline (489us):
  - Input is pre-cast to bf16 host-side (1MB instead of 2MB on the
    critical path; the f32 residual spine is rebuilt from bf16).
  - Attention is head-major and software-pipelined: QK(i+1) issues
    before AV(i) so the PE has work under the ACT exp shadow.
  - exp is batched 4 k-blocks per activation ([128,1024], amortizes the
    ~352-cycle ACT fixed cost) and writes fp8 directly.
  - Causal masks are ADDITIVE on the PSUM scores before exp (so exp's
    fp8 output needs no DVE post-pass).
  - AV and the V-projection run in fp8 DoubleRow mode (2 contraction
    chunks per matmul).
  - LN2 is folded into fc1 (gamma/beta host-folded, mean*rstd via a
    2-row correction matmul per chain), which also lets gelu batch over
    j-pairs ([128,1024] per activation).
  - Softmax denominators use the DVE fast reciprocal (no ACT table swap).
"""

import numpy as np
import ml_dtypes

P = 128
B, S, D, H = 2, 2048, 1024, 16
DH = D // H          # 64
DI = 4 * D           # 4096
EPS = 1e-5
NCORES = 8
TT = B * S           # 4096 flat tokens
TOK = TT // NCORES   # 512 tokens per core
KD = D // P          # 8
KDI = DI // P        # 32
QCH = 256            # query chunk (2 blocks of 128)
NQC = S // QCH       # 8 query chunks per batch
HL = H // NCORES     # 2 local heads
RG = [list(range(NCORES))]
DGR = D + 2          # gathered rows: D of xs, mu*rstd row, ones row
VP = 80              # padded v free-dim (65 used; 80 keeps DR stride %16)

_CACHED_NC = None


def build_nc():
    import concourse.bacc as bacc
    import concourse.tile as tile
    import concourse.mybir as mybir
    from contextlib import ExitStack

    dt = mybir.dt
    f32, bf16, f32r = dt.float32, dt.bfloat16, dt.float32r
    fp8 = dt.float8e4
    DR = mybir.MatmulPerfMode.DoubleRow
    AF = mybir.ActivationFunctionType
    OP = mybir.AluOpType

    nc = bacc.Bacc("TRN2", target_bir_lowering=False, debug=False,
                   num_devices=NCORES)

    # ---- kernel I/O (per-core shapes) ----
    xTb = nc.dram_tensor("xTb", [D, TOK], bf16, kind="ExternalInput").ap()
    awg = nc.dram_tensor("awg", [P, KD, 3 * P], fp8, kind="ExternalInput").ap()
    cr = nc.dram_tensor("cr", [2, 3, P], fp8, kind="ExternalInput").ap()
    pw = nc.dram_tensor("pw", [KD, P, KD, P], fp8, kind="ExternalInput").ap()
    pb = nc.dram_tensor("pb", [P, KD], f32, kind="ExternalInput").ap()
    fw = nc.dram_tensor("fw", [KDI // 2, 2, P, KD, P], bf16,
                        kind="ExternalInput").ap()
    fcc = nc.dram_tensor("fcc", [2, KDI, P], bf16, kind="ExternalInput").ap()
    gw = nc.dram_tensor("gw", [KD, 4, P, KDI // 4, P], bf16,
                        kind="ExternalInput").ap()
    gb = nc.dram_tensor("gb", [P, KD], f32, kind="ExternalInput").ap()
    mk = nc.dram_tensor("mk", [P, 2, QCH], bf16, kind="ExternalInput").ap()
    outT = nc.dram_tensor("outT", [D, TOK], f32, kind="ExternalOutput").ap()

    def act_raw(out, in_, func, bias=0.0, scale=1.0):
        """nc.scalar.activation minus the Reciprocal/Rsqrt accuracy guard."""
        se = nc.scalar
        ins = [se.lower_ap(in_)]
        for arg in (bias, scale, 0.0):
            if isinstance(arg, float):
                ins.append(mybir.ImmediateValue(dtype=f32, value=arg))
            else:
                ins.append(se.lower_ap(arg))
        return se.add_instruction(mybir.InstActivation(
            name=se.bass.get_next_instruction_name(),
            func=func, ins=ins, outs=[se.lower_ap(out)]))

    with tile.TileContext(nc) as tc, ExitStack() as ctx:
        const = ctx.enter_context(tc.tile_pool(name="const", bufs=1))
        dram = ctx.enter_context(tc.tile_pool(name="dram", bufs=1, space="DRAM"))
        psum = ctx.enter_context(tc.tile_pool(name="psum", bufs=1, space="PSUM"))
        rows = ctx.enter_context(tc.tile_pool(name="rows", bufs=6))
        sqp = ctx.enter_context(tc.tile_pool(name="sqp", bufs=2))
        lnt = ctx.enter_context(tc.tile_pool(name="lnt", bufs=3))
        res = ctx.enter_context(tc.tile_pool(name="res", bufs=1))

        # ---- input + constants ----
        xv = xTb.rearrange("(k p) c -> p k c", p=P)

        awg_sb = const.tile([P, KD, 3 * P], fp8)
        cr_sb = const.tile([2, 3, P], fp8)
        mka = const.tile([P, 2, QCH], bf16)
        pb_sb = const.tile([P, KD], f32)
        fcc_sb = const.tile([2, KDI, P], bf16)
        gb_sb = const.tile([P, KD], f32)
        ones_cb = const.tile([P, 1], bf16)
        nc.vector.memset(ones_cb, 1.0)
        ones_tok_b = const.tile([1, TOK], bf16)
        nc.vector.memset(ones_tok_b, 1.0)
        ones_tok = const.tile([1, TOK], fp8)
        nc.vector.tensor_copy(ones_tok, ones_tok_b)
        ones_rf = const.tile([1, P], f32)
        nc.vector.memset(ones_rf, 1.0)
        ones_r = const.tile([1, P], f32r)
        nc.vector.tensor_copy(ones_r, ones_rf)
        ones_cf = const.tile([P, 1], f32)
        nc.vector.memset(ones_cf, 1.0)
        ones_c = const.tile([P, 1], f32r)
        nc.vector.tensor_copy(ones_c, ones_cf)
        eps_sb = const.tile([1, 1], f32)
        nc.vector.memset(eps_sb, EPS)
        mr2ones = const.tile([2, TOK], bf16)
        nc.vector.memset(mr2ones, 1.0)  # row 0 overwritten with mu2*rstd2

        # collective bounce buffers; cc1 split so the first AllGather can
        # fire after xs chunks 0-3 and its successor's transfer overlaps
        # the prefetch DMAs + rank-0 first-half QKV chains
        cc1a_in = dram.tile([D // 2, TOK], fp8)
        cc1a_out = dram.tile([NCORES, D // 2, TOK], fp8, addr_space="Shared")
        cc1b_in = dram.tile([D // 2 + 2, TOK], fp8)
        cc1b_out = dram.tile([NCORES, D // 2 + 2, TOK], fp8,
                             addr_space="Shared")
        cc2_in = [dram.tile([NCORES, DH + 1, TOK], bf16, name=f"c2i{h}")
                  for h in range(HL)]
        cc2_out = [dram.tile([NCORES, DH + 1, TOK], bf16, name=f"c2o{h}")
                   for h in range(HL)]

        # ---- phase 1: LN1 stats, xs = x*rstd (fp8), mr row -> AllGather ----
        s0 = ExitStack()
        psS = s0.enter_context(tc.tile_pool(name="psS", bufs=1, space="PSUM"))
        lnw = s0.enter_context(tc.tile_pool(name="lnw", bufs=2))
        xb = lnw.tile([P, KD, TOK], bf16, tag="xb", bufs=1)
        for k in range(KD):
            nc.sync.dma_start(xb[:, k, 0:TOK // 2], xv[:, k, 0:TOK // 2])
            nc.sync.dma_start(xb[:, k, TOK // 2:], xv[:, k, TOK // 2:])
        # const DMAs issue after the critical-path input chunks
        nc.sync.dma_start(awg_sb, awg)
        nc.sync.dma_start(cr_sb, cr)
        nc.sync.dma_start(mka, mk)
        nc.sync.dma_start(pb_sb, pb)
        nc.sync.dma_start(fcc_sb, fcc)
        nc.sync.dma_start(gb_sb, gb)
        with nc.named_scope("ln1"):
            sx1 = psS.tile([1, TOK], f32, tag="s1")
            sx2 = psS.tile([1, TOK], f32, tag="s2")
            for k in range(KD):
                sq = lnw.tile([P, TOK], bf16, tag="sq")
                nc.vector.tensor_mul(sq, xb[:, k, :], xb[:, k, :])
                nc.tensor.matmul(sx1, ones_cb, xb[:, k, :],
                                 start=(k == 0), stop=(k == KD - 1),
                                 skip_group_check=True)
                nc.tensor.matmul(sx2, ones_cb, sq,
                                 start=(k == 0), stop=(k == KD - 1),
                                 skip_group_check=True)
            mu = rows.tile([1, TOK], f32, tag="row")
            nc.vector.tensor_scalar_mul(mu, sx1, 1.0 / D)
            m2 = rows.tile([1, TOK], f32, tag="row")
            nc.vector.tensor_scalar_mul(m2, sx2, 1.0 / D)
            var = rows.tile([1, TOK], f32, tag="row")
            nc.vector.tensor_tensor(out=var, in0=mu, in1=mu, op=OP.mult)
            nc.vector.tensor_tensor(out=var, in0=m2, in1=var, op=OP.subtract)
            rstd = rows.tile([1, TOK], f32r, tag="row")
            act_raw(rstd, var, AF.Rsqrt, bias=eps_sb[:])
            mr = rows.tile([1, TOK], fp8, tag="mrow")
            nc.vector.tensor_tensor(out=mr, in0=mu, in1=rstd, op=OP.mult)
            nc.sync.dma_start(cc1b_in[D // 2:D // 2 + 1, :], mr)
            nc.sync.dma_start(cc1b_in[D // 2 + 1:D // 2 + 2, :], ones_tok)
            rstd_b = psum.tile([P, TOK], f32, tag="big", bufs=2)
            nc.tensor.matmul(rstd_b, ones_r, rstd, start=True, stop=True)
            cc1av = cc1a_in[:].rearrange("(k p) c -> p k c", p=P)
            cc1bv = cc1b_in[0:D // 2, :].rearrange("(k p) c -> p k c", p=P)
            rstd_s = lnw.tile([P, TOK], f32, tag="rs", bufs=1)
            nc.vector.tensor_copy(rstd_s, rstd_b)
            xs8 = lnw.tile([P, KD, TOK], fp8, tag="x8", bufs=1)
            for k in range(KD):
                eng = nc.vector if k % 2 == 0 else nc.gpsimd
                eng.tensor_tensor(out=xs8[:, k, :], in0=xb[:, k, :],
                                  in1=rstd_s, op=OP.mult)
                dst = cc1av[:, k, :] if k < 4 else cc1bv[:, k - 4, :]
                nc.sync.dma_start(dst, xs8[:, k, :])
                if k == 3:
                    with nc.named_scope("ag"):
                        nc.gpsimd.collective_compute(
                            "AllGather", OP.bypass, replica_groups=RG,
                            ins=[cc1a_in[:].opt()],
                            outs=[cc1a_out[:].opt()])
        with nc.named_scope("ag2"):
            nc.gpsimd.collective_compute(
                "AllGather", OP.bypass, replica_groups=RG,
                ins=[cc1b_in[:].opt()], outs=[cc1b_out[:].opt()])
        s0.close()  # free the LN1 stats PSUM banks for the attention pools

        att2 = ctx.enter_context(tc.tile_pool(name="att2", bufs=1))
        # attention-era pools (freed before the MLP pools allocate)
        s1 = ExitStack()
        ares = s1.enter_context(tc.tile_pool(name="ares", bufs=1))
        xnp = s1.enter_context(tc.tile_pool(name="xnp", bufs=1))
        wp = s1.enter_context(tc.tile_pool(name="wp", bufs=3))
        psA = s1.enter_context(tc.tile_pool(name="psA", bufs=2, space="PSUM"))

        # ---- phase 2: QKV (fp8 DoubleRow) for local heads, all tokens ----
        kT = [ares.tile([P, TOK], bf16, name=f"kT{r}") for r in range(NCORES)]
        qT = [ares.tile([P, TOK], bf16, name=f"qT{r}") for r in range(NCORES)]
        vA = [ares.tile([P, 4, HL, VP], fp8, name=f"vA{r}")
              for r in range(NCORES)]
        for r in range(NCORES):
            nc.vector.memset(vA[r][:, :, :, DH:DH + 1], 1.0)

        # gathered activations: 6-slot ring, prefetches staged so ranks 4-7
        # stream in under the attention exp shadow
        xs_t = {}
        mro_r = [xnp.tile([2, TOK], fp8, name=f"mro{r}")
                 for r in range(NCORES)]

        def prefetch(r):
            xs_t[r] = xnp.tile([P, KD, TOK], fp8, tag="xsr", bufs=5,
                               name=f"xsr{r}")
            # per-chunk DMAs spread across the 16 DMA engines; chunks 0-3
            # come from the first AllGather and land while AG2 transfers
            cva = cc1a_out[r].rearrange("(k p) c -> p k c", p=P)
            cvb = cc1b_out[r, 0:D // 2, :].rearrange("(k p) c -> p k c", p=P)
            for k in range(4):
                nc.sync.dma_start(xs_t[r][:, k, :], cva[:, k, :])
            for k in range(4):
                nc.sync.dma_start(xs_t[r][:, 4 + k, :], cvb[:, k, :])
            nc.sync.dma_start(mro_r[r], cc1b_out[r, D // 2:D // 2 + 2, :])

        def emit_qk(r):
            for which in range(2):  # 0 -> q, 1 -> k
                ps = psum.tile([P, TOK], f32, tag="big", bufs=2)
                cb = which * P
                for kp in range(KD // 2):
                    nc.tensor.matmul(ps, awg_sb[:, 2 * kp:2 * kp + 2,
                                                cb:cb + P],
                                     xs_t[r][:, 2 * kp:2 * kp + 2, :],
                                     perf_mode=DR,
                                     start=(kp == 0), stop=False)
                nc.tensor.matmul(ps, cr_sb[:, which, :], mro_r[r],
                                 start=False, stop=True)
                dst = qT[r] if which == 0 else kT[r]
                nc.vector.tensor_scalar_mul(dst, ps, 1.0 / 16)

        def emit_v(r):
            for t in range(4):
                psv = psum.tile([P, P], f32, tag="big", bufs=2)
                for kp in range(KD // 2):
                    nc.tensor.matmul(psv,
                                     xs_t[r][:, 2 * kp:2 * kp + 2,
                                             P * t:P * (t + 1)],
                                     awg_sb[:, 2 * kp:2 * kp + 2, 2 * P:3 * P],
                                     perf_mode=DR,
                                     start=(kp == 0), stop=False)
                nc.tensor.matmul(psv, mro_r[r][:, P * t:P * (t + 1)],
                                 cr_sb[:, 2, :], start=False, stop=True)
                nc.vector.tensor_scalar_mul(
                    vA[r][:, t, :, 0:DH],
                    psv.rearrange("p (h d) -> p h d", h=HL), 1.0 / 16)

        # ---- phase 3: causal attention, head-major, software-pipelined ----
        aT = ares.tile([P, NCORES, TOK], bf16)
        drows = ares.tile([1, HL, NCORES, TOK], bf16)
        aFb = [att2.tile([P, TOK], bf16, name=f"aFb{r}")
               for r in range(NCORES)]
        aF8 = att2.tile([P, NCORES, TOK], fp8)
        d2 = [att2.tile([1, HL, TOK], bf16, name=f"d2{r}")
              for r in range(NCORES)]

        def ship(h, j):
            nc.sync.dma_start(cc2_in[h][j][0:DH], aT[DH * h:DH * h + DH, j, :])
            nc.sync.dma_start(cc2_in[h][j][DH:DH + 1],
                              drows[0:1, h, j, :])

        def fetch_back(h, r):
            nc.sync.dma_start(aFb[r][DH * h:DH * h + DH, :],
                              cc2_out[h][r][0:DH])
            nc.sync.dma_start(d2[r][0:1, h, :], cc2_out[h][r][DH:DH + 1])

        def attn_head(h, boundary_work):
            """boundary_work: dict (b, qc) -> [thunks] emitted after that
            query-chunk's last AV group is issued (PE filler under exp)."""
            hb = DH * h
            items = []
            for b in (0, 1):
                for qc in range(NQC):
                    nkb = 2 * qc + 2
                    kb0 = 0
                    while kb0 < nkb:
                        nblk = min(4, nkb - kb0)
                        items.append((b, qc, kb0, nblk))
                        kb0 += nblk
            accs = {}
            pending = None

            def emit_av(w8, it):
                b, qc, kb0, nblk = it
                nkb = 2 * qc + 2
                acc = accs[(b, qc)]
                for pr in range(nblk // 2):
                    kb = kb0 + 2 * pr
                    r = 4 * b + kb // 4
                    t = kb % 4
                    nc.tensor.matmul(acc, vA[r][:, t:t + 2, h, 0:DH + 1],
                                     w8[:, 2 * pr:2 * pr + 2, :],
                                     perf_mode=DR,
                                     start=(kb0 == 0 and pr == 0),
                                     stop=(kb + 2 == nkb),
                                     skip_group_check=True)
                if kb0 + nblk == nkb:  # last group of this query chunk
                    qr = 4 * b + qc // 2
                    qo = QCH * (qc % 2)
                    nc.vector.tensor_copy(aT[hb:hb + DH, qr, qo:qo + QCH],
                                          acc[0:DH, :])
                    nc.vector.tensor_copy(drows[0:1, h, qr, qo:qo + QCH],
                                          acc[DH:DH + 1, :])
                    del accs[(b, qc)]
                    if qc % 2 == 1:
                        ship(h, qr)
                    for thunk in boundary_work.get((b, qc), ()):
                        thunk()

            for it in items:
                b, qc, kb0, nblk = it
                qr = 4 * b + qc // 2
                qo = QCH * (qc % 2)
                if kb0 == 0:
                    accs[(b, qc)] = psA.tile([DH + 1, QCH], f32, tag="acc",
                                             bufs=2, name=f"acc{h}_{b}_{qc}")
                sc = psA.tile([P, 4, QCH], f32, tag="sc", bufs=2)
                for j in range(nblk):
                    kb = kb0 + j
                    r = 4 * b + kb // 4
                    t = kb % 4
                    nc.tensor.matmul(sc[:, j, :],
                                     kT[r][hb:hb + DH, P * t:P * (t + 1)],
                                     qT[qr][hb:hb + DH, qo:qo + QCH],
                                     start=True, stop=True,
                                     skip_group_check=True)
                if kb0 + nblk == 2 * qc + 2:  # diagonal pair: additive mask
                    dj = nblk - 2
                    nc.vector.tensor_tensor(out=sc[:, dj:dj + 2, :],
                                            in0=sc[:, dj:dj + 2, :],
                                            in1=mka, op=OP.add)
                w8 = wp.tile([P, 4, QCH], fp8, tag="w")
                nc.scalar.activation(w8[:, 0:nblk, :], sc[:, 0:nblk, :],
                                     AF.Exp, scale=0.125)
                if pending is not None:
                    emit_av(*pending)
                pending = (w8, it)
            emit_av(*pending)

        with nc.named_scope("qkv"):
            for r in range(4):
                prefetch(r)
            emit_qk(0)
            emit_v(0)

        # PE filler schedule: rank r's qk/v must exist before the first
        # query chunk that reads k-block 4*(r%4) of batch r//4. Prefetches
        # for ranks 4-7 stream in a few chunks ahead (6-slot ring).
        bw0 = {
            (0, 0): [lambda: emit_qk(1)],
            (0, 1): [lambda: emit_v(1), lambda: emit_qk(2),
                     lambda: prefetch(4)],
            (0, 2): [lambda: emit_v(2), lambda: prefetch(5)],
            (0, 3): [lambda: emit_qk(3), lambda: emit_v(3),
                     lambda: prefetch(6)],
            (0, 4): [lambda: emit_qk(4), lambda: emit_v(4),
                     lambda: prefetch(7)],
            (0, 5): [lambda: emit_qk(5), lambda: emit_v(5)],
            (0, 6): [lambda: emit_qk(6), lambda: emit_v(6)],
            (0, 7): [lambda: emit_qk(7), lambda: emit_v(7)],
        }
        with nc.named_scope("attn0"):
            attn_head(0, bw0)
        with nc.named_scope("a2a0"):
            nc.gpsimd.collective_compute(
                "AllToAll", OP.bypass, replica_groups=RG,
                ins=[cc2_in[0][:].opt()], outs=[cc2_out[0][:].opt()])
        # head 1: interleave head-0 result prefetches under the exp shadow
        bw1 = {(0, qc): [lambda r=qc: fetch_back(0, r)] for qc in range(NQC)}
        with nc.named_scope("attn1"):
            attn_head(1, bw1)
        with nc.named_scope("a2a1"):
            nc.gpsimd.collective_compute(
                "AllToAll", OP.bypass, replica_groups=RG,
                ins=[cc2_in[1][:].opt()], outs=[cc2_out[1][:].opt()])
        for r in range(NCORES):
            fetch_back(1, r)

        s1.close()  # release attention-era SBUF/PSUM
        s2 = ExitStack()
        psB = s2.enter_context(tc.tile_pool(name="psB", bufs=2, space="PSUM"))
        mlp = ctx.enter_context(tc.tile_pool(name="mlp", bufs=1))
        wgt = ctx.enter_context(tc.tile_pool(name="wgt", bufs=1))
        outp = ctx.enter_context(tc.tile_pool(name="outp", bufs=1))

        # ---- phase 4: normalize by softmax denominators + output proj ----
        h1T = mlp.tile([P, KD, TOK], f32)
        xbr = mlp.tile([P, KD, TOK], bf16)
        for k in range(KD):
            nc.sync.dma_start(xbr[:, k, :], xv[:, k, :])
        st2b = psB.tile([1, TOK], f32, tag="s1", bufs=1)
        st2c = psB.tile([1, TOK], f32, tag="s2", bufs=1)
        with nc.named_scope("proj"):
            # head-0 halves normalize while the head-1 AllToAll is still in
            # flight (their data arrived during attn1); head-1 after.
            # di carries a x32 scale so the fp8 aF8 uses its dynamic range
            # (descale folded into the proj drain).
            for h in range(HL):
                for r in range(NCORES):
                    di = outp.tile([1, TOK], bf16, tag="di", bufs=2)
                    act_raw(di, d2[r][0:1, h, :], AF.Reciprocal,
                            scale=1.0 / 32)
                    dbc = psB.tile([P, TOK], f32, tag="bc", bufs=2)
                    nc.tensor.matmul(dbc[DH * h:DH * h + DH, :],
                                     ones_tok_b[:, 0:DH], di,
                                     start=True, stop=True,
                                     skip_group_check=True)
                    nc.vector.tensor_tensor(
                        out=aF8[DH * h:DH * h + DH, r, :],
                        in0=aFb[r][DH * h:DH * h + DH, :],
                        in1=dbc[DH * h:DH * h + DH, :], op=OP.mult)
            for f in range(KD):
                pwt = wgt.tile([P, KD, P], fp8, tag="pw", bufs=2)
                nc.sync.dma_start(pwt, pw[f])
                ps = psum.tile([P, TOK], f32, tag="big", bufs=2)
                for k in range(KD // 2):
                    nc.tensor.matmul(ps, pwt[:, 2 * k:2 * k + 2, :],
                                     aF8[:, 2 * k:2 * k + 2, :],
                                     perf_mode=DR,
                                     start=(k == 0), stop=(k == KD // 2 - 1))
                t1 = lnt.tile([P, TOK], f32, tag="pj")
                nc.vector.tensor_scalar(out=t1, in0=ps,
                                        scalar1=1.0 / 512,
                                        scalar2=pb_sb[:, f:f + 1],
                                        op0=OP.mult, op1=OP.add)
                nc.vector.tensor_tensor(out=h1T[:, f, :], in0=t1,
                                        in1=xbr[:, f, :], op=OP.add)
                xr = sqp.tile([P, TOK], f32r, tag="xr")
                nc.vector.tensor_copy(xr, h1T[:, f, :])
                sq = sqp.tile([P, TOK], f32r, tag="sq2")
                nc.vector.tensor_mul(sq, h1T[:, f, :], h1T[:, f, :])
                nc.tensor.matmul(st2b, ones_c, xr,
                                 start=(f == 0), stop=(f == KD - 1),
                                 skip_group_check=True)
                nc.tensor.matmul(st2c, ones_c, sq,
                                 start=(f == 0), stop=(f == KD - 1),
                                 skip_group_check=True)

        # ---- phase 5: LN2 (folded into fc1) -> xs2 = h1*rstd2 (bf16) ----
        xs2 = mlp.tile([P, KD, TOK], bf16)
        with nc.named_scope("ln2"):
            mu2 = rows.tile([1, TOK], f32, tag="row")
            nc.vector.tensor_scalar_mul(mu2, st2b, 1.0 / D)
            m22 = rows.tile([1, TOK], f32, tag="row")
            nc.vector.tensor_scalar_mul(m22, st2c, 1.0 / D)
            var2 = rows.tile([1, TOK], f32, tag="row")
            nc.vector.tensor_tensor(out=var2, in0=mu2, in1=mu2, op=OP.mult)
            nc.vector.tensor_tensor(out=var2, in0=m22, in1=var2,
                                    op=OP.subtract)
            rstd2 = rows.tile([1, TOK], f32r, tag="row")
            act_raw(rstd2, var2, AF.Rsqrt, bias=eps_sb[:])
            nc.vector.tensor_tensor(out=mr2ones[0:1, :], in0=mu2, in1=rstd2,
                                    op=OP.mult)
            rsb = psB.tile([P, TOK], f32, tag="bc", bufs=2)
            nc.tensor.matmul(rsb, ones_r, rstd2, start=True, stop=True)
            rsb_s = lnt.tile([P, TOK], f32, tag="pj")
            nc.vector.tensor_copy(rsb_s, rsb)
            for k in range(KD):
                eng = nc.vector if k % 2 == 0 else nc.gpsimd
                eng.tensor_tensor(out=xs2[:, k, :], in0=h1T[:, k, :],
                                  in1=rsb_s, op=OP.mult)

        s2.close()  # free proj/LN2 PSUM before the fc1 pool opens
        s3 = ExitStack()
        psF = s3.enter_context(tc.tile_pool(name="psF", bufs=2, space="PSUM"))

        # ---- phase 6: MLP (LN2-folded fc1 + batched gelu, then fc2) ----
        hT = mlp.tile([P, KDI, TOK], bf16)
        # hoist the first weight tile's DMA so it streams in during ln2
        def fw_tile(jp, half):
            t = wgt.tile([P, KD, P], bf16, tag="fw", bufs=3,
                         name=f"fwt{jp}_{half}")
            nc.sync.dma_start(t, fw[jp, half])
            return t
        fwt0 = fw_tile(0, 0)
        with nc.named_scope("fc1"):
            for jp in range(KDI // 2):
                ps = psF.tile([P, 2, TOK], f32, tag="f1", bufs=2)
                for half in range(2):
                    j = 2 * jp + half
                    fwt = fwt0 if j == 0 else fw_tile(jp, half)
                    for k in range(KD):
                        nc.tensor.matmul(ps[:, half, :], fwt[:, k, :],
                                         xs2[:, k, :],
                                         start=(k == 0), stop=False,
                                         skip_group_check=True)
                    nc.tensor.matmul(ps[:, half, :], fcc_sb[:, j, :],
                                     mr2ones, start=False, stop=True,
                                     skip_group_check=True)
                nc.scalar.activation(hT[:, 2 * jp:2 * jp + 2, :], ps,
                                     AF.Gelu_apprx_tanh)
        with nc.named_scope("fc2"):
            KH = KDI // 4
            for f in range(KD):
                ps = psum.tile([P, TOK], f32, tag="big", bufs=2)
                for half in range(4):
                    gwt = wgt.tile([P, KH, P], bf16, tag="gw", bufs=3)
                    nc.sync.dma_start(gwt, gw[f, half])
                    for kk in range(KH):
                        k = KH * half + kk
                        nc.tensor.matmul(ps, gwt[:, kk, :], hT[:, k, :],
                                         start=(k == 0),
                                         stop=(k == KDI - 1))
                o = outp.tile([P, TOK], f32, tag="ot", bufs=2)
                nc.vector.tensor_scalar_add(o, ps, gb_sb[:, f:f + 1])
                nc.vector.tensor_tensor(out=o, in0=o, in1=h1T[:, f, :],
                                        op=OP.add)
                nc.sync.dma_start(outT[P * f:P * (f + 1), :], o)
        s3.close()

    nc.compile()
    return nc


def shard_inputs(inputs):
    """Full inputs -> list of 8 per-core input dicts (host-side layout only)."""
    bf16 = ml_dtypes.bfloat16
    f32 = np.float32
    f8 = ml_dtypes.float8_e4m3fn
    hs = np.asarray(inputs["hidden_states"], f32).reshape(TT, D)
    attn_w = np.asarray(inputs["attn_w"], f32)
    attn_b = np.asarray(inputs["attn_b"], f32)
    l1g = np.asarray(inputs["ln1_g"], f32)
    l1b = np.asarray(inputs["ln1_b"], f32)
    l2g = np.asarray(inputs["ln2_g"], f32)
    l2b = np.asarray(inputs["ln2_b"], f32)

    def col(v):  # [D] -> [P, KD]
        return np.ascontiguousarray(np.asarray(v, f32).reshape(KD, P).T)

    pw = np.ascontiguousarray((np.asarray(inputs["proj_w"], f32) * 16)
                              .reshape(KD, P, KD, P).transpose(2, 1, 0, 3)
                              .astype(f8))
    # fc1: fold LN2 gamma into the weights; beta@W + fc_b and the colsum
    # correction go into the 2-row fcc tensor. Pair layout for batched gelu.
    fw_full = np.asarray(inputs["fc_w"], f32)
    fwg = fw_full * l2g[:, None]
    c1 = fwg.sum(axis=0)                              # [DI]
    fbe = l2b @ fw_full + np.asarray(inputs["fc_b"], f32)
    fcc = np.ascontiguousarray(
        np.stack([-c1, fbe]).reshape(2, KDI, P).astype(bf16))
    fw = np.ascontiguousarray(
        fwg.reshape(KD, P, KDI // 2, 2, P).transpose(2, 3, 1, 0, 4)
        .astype(bf16))
    gw = np.ascontiguousarray(np.asarray(inputs["fc2_w"], f32)
                              .reshape(4, KDI // 4, P, KD, P)
                              .transpose(3, 0, 2, 1, 4)
                              .astype(bf16))
    pb = col(inputs["proj_b"])
    gbv = col(inputs["fc2_b"])

    # additive causal masks for the diagonal k-block pair, pre-exp on PSUM
    # (exp applies scale 0.125 to score+mask, so -1e5 -> exp -> 0)
    ii, jj = np.meshgrid(np.arange(P), np.arange(QCH), indexing="ij")
    mk = np.stack([np.where(jj >= ii, 0.0, -1e5),
                   np.where(jj >= ii + P, 0.0, -1e5)], axis=1).astype(bf16)
    mk = np.ascontiguousarray(mk)                     # [P, 2, QCH]

    maps = []
    for c in range(NCORES):
        cols = np.r_[P * c:P * (c + 1),
                     D + P * c:D + P * (c + 1),
                     2 * D + P * c:2 * D + P * (c + 1)]
        w_c = attn_w[:, cols]                      # [D, 384]
        wg_c = w_c * l1g[:, None]                  # gamma folded
        c1a = wg_c.sum(axis=0)                     # [384]
        cba = l1b @ w_c + attn_b[cols]             # [384]
        cr_c = np.ascontiguousarray((np.stack([
            np.stack([-c1a[0:P], -c1a[P:2 * P], -c1a[2 * P:3 * P]]),
            np.stack([cba[0:P], cba[P:2 * P], cba[2 * P:3 * P]]),
        ]) * 16).astype(f8))
        awg_c = np.ascontiguousarray(
            (wg_c * 16).reshape(KD, P, 3 * P).transpose(1, 0, 2).astype(f8))
        xTb_c = np.ascontiguousarray(hs[TOK * c:TOK * (c + 1)].T.astype(bf16))
        maps.append({
            "xTb": xTb_c, "awg": awg_c, "cr": cr_c,
            "pw": pw, "pb": pb, "fw": fw, "fcc": fcc, "gw": gw, "gb": gbv,
            "mk": mk,
        })
    return maps


def unshard(results):
    out = np.concatenate([np.asarray(r["outT"]).T for r in results], axis=0)
    return np.ascontiguousarray(out.reshape(B, S, D))


def kernel(**inputs):
    global _CACHED_NC
    from concourse.bass_utils import run_bass_kernel_spmd
    if _CACHED_NC is None:
        _CACHED_NC = build_nc()
    in_maps = shard_inputs(inputs)
    res = run_bass_kernel_spmd(_CACHED_NC, in_maps,
                               core_ids=list(range(NCORES)))
    return unshard(res.results)


# revision 56
# speedup vs baseline: 1.0904x; 1.0904x over previous
"""Trainium2 Bass kernel for a GPT-2 style transformer block (nn_Block_16690242913196).

Sharding (8 NeuronCores, identical SPMD program):
  - QKV/proj/MLP: token-parallel (core i owns 512 flat tokens).
  - Attention: head-parallel (core i owns heads {2i, 2i+1} for ALL tokens).
  - Collective 1: AllGather of rstd-scaled activations (+ mu*rstd row,
    ones row) in fp8. LN1 is folded into the QKV matmuls via rank-1
    correction rows appended to each PSUM chain.
  - Collectives 2a/2b: per-head AllToAll of attention outputs (+ the
    softmax-denominator row). Head 0's AllToAll fires at ~50% of the
    attention phase and hides under head 1's compute.

Perf structure vs the original baseline (489us):
  - Input is pre-cast to bf16 host-side (1MB instead of 2MB on the
    critical path; the f32 residual spine is rebuilt from bf16).
  - Attention is head-major and software-pipelined: QK(i+1) issues
    before AV(i) so the PE has work under the ACT exp shadow.
  - exp is batched 4 k-blocks per activation ([128,1024], amortizes the
    ~352-cycle ACT fixed cost) and writes fp8 directly.
  - Causal masks are ADDITIVE on the PSUM scores before exp (so exp's
    fp8 output needs no DVE post-pass).
  - AV and the V-projection run in fp8 DoubleRow mode (2 contraction
    chunks per matmul).
  - LN2 is folded into fc1 (gamma/beta host-folded, mean*rstd via a
    2-row correction matmul per chain), which also lets gelu batch over
    j-pairs ([128,1024] per activation).
  - Softmax denominators use the DVE fast reciprocal (no ACT table swap).
"""

import numpy as np
import ml_dtypes

P = 128
B, S, D, H = 2, 2048, 1024, 16
DH = D // H          # 64
DI = 4 * D           # 4096
EPS = 1e-5
NCORES = 8
TT = B * S           # 4096 flat tokens
TOK = TT // NCORES   # 512 tokens per core
KD = D // P          # 8
KDI = DI // P        # 32
QCH = 256            # query chunk (2 blocks of 128)
NQC = S // QCH       # 8 query chunks per batch
HL = H // NCORES     # 2 local heads
RG = [list(range(NCORES))]
DGR = D + 2          # gathered rows: D of xs, mu*rstd row, ones row
VP = 80              # padded v free-dim (65 used; 80 keeps DR stride %16)

_CACHED_NC = None


def build_nc():
    import concourse.bacc as bacc
    import concourse.tile as tile
    import concourse.mybir as mybir
    from contextlib import ExitStack

    dt = mybir.dt
    f32, bf16, f32r = dt.float32, dt.bfloat16, dt.float32r
    fp8 = dt.float8e4
    DR = mybir.MatmulPerfMode.DoubleRow
    AF = mybir.ActivationFunctionType
    OP = mybir.AluOpType

    nc = bacc.Bacc("TRN2", target_bir_lowering=False, debug=False,
                   num_devices=NCORES)

    # ---- kernel I/O (per-core shapes) ----
    xTb = nc.dram_tensor("xTb", [D, TOK], bf16, kind="ExternalInput").ap()
    awg = nc.dram_tensor("awg", [P, KD, 3 * P], fp8, kind="ExternalInput").ap()
    cr = nc.dram_tensor("cr", [2, 3, P], fp8, kind="ExternalInput").ap()
    pw = nc.dram_tensor("pw", [KD, P, KD, P], fp8, kind="ExternalInput").ap()
    pb = nc.dram_tensor("pb", [P, KD], f32, kind="ExternalInput").ap()
    fw = nc.dram_tensor("fw", [KDI // 2, 2, P, KD, P], bf16,
                        kind="ExternalInput").ap()
    fcc = nc.dram_tensor("fcc", [2, KDI, P], bf16, kind="ExternalInput").ap()
    gw = nc.dram_tensor("gw", [KD, 4, P, KDI // 4, P], bf16,
                        kind="ExternalInput").ap()
    gb = nc.dram_tensor("gb", [P, KD], f32, kind="ExternalInput").ap()
    mk = nc.dram_tensor("mk", [P, 2, QCH], bf16, kind="ExternalInput").ap()
    outT = nc.dram_tensor("outT", [D, TOK], f32, kind="ExternalOutput").ap()

    def act_raw(out, in_, func, bias=0.0, scale=1.0):
        """nc.scalar.activation minus the Reciprocal/Rsqrt accuracy guard."""
        se = nc.scalar
        ins = [se.lower_ap(in_)]
        for arg in (bias, scale, 0.0):
            if isinstance(arg, float):
                ins.append(mybir.ImmediateValue(dtype=f32, value=arg))
            else:
                ins.append(se.lower_ap(arg))
        return se.add_instruction(mybir.InstActivation(
            name=se.bass.get_next_instruction_name(),
            func=func, ins=ins, outs=[se.lower_ap(out)]))

    with tile.TileContext(nc) as tc, ExitStack() as ctx:
        const = ctx.enter_context(tc.tile_pool(name="const", bufs=1))
        dram = ctx.enter_context(tc.tile_pool(name="dram", bufs=1, space="DRAM"))
        psum = ctx.enter_context(tc.tile_pool(name="psum", bufs=1, space="PSUM"))
        rows = ctx.enter_context(tc.tile_pool(name="rows", bufs=6))
        sqp = ctx.enter_context(tc.tile_pool(name="sqp", bufs=2))
        lnt = ctx.enter_context(tc.tile_pool(name="lnt", bufs=3))
        res = ctx.enter_context(tc.tile_pool(name="res", bufs=1))

        # ---- input + constants ----
        xv = xTb.rearrange("(k p) c -> p k c", p=P)

        awg_sb = const.tile([P, KD, 3 * P], fp8)
        cr_sb = const.tile([2, 3, P], fp8)
        mka = const.tile([P, 2, QCH], bf16)
        pb_sb = const.tile([P, KD], f32)
        fcc_sb = const.tile([2, KDI, P], bf16)
        gb_sb = const.tile([P, KD], f32)
        ones_cb = const.tile([P, 1], bf16)
        nc.vector.memset(ones_cb, 1.0)
        ones_tok_b = const.tile([1, TOK], bf16)
        nc.vector.memset(ones_tok_b, 1.0)
        ones_tok = const.tile([1, TOK], fp8)
        nc.vector.tensor_copy(ones_tok, ones_tok_b)
        ones_rf = const.tile([1, P], f32)
        nc.vector.memset(ones_rf, 1.0)
        ones_r = const.tile([1, P], f32r)
        nc.vector.tensor_copy(ones_r, ones_rf)
        ones_cf = const.tile([P, 1], f32)
        nc.vector.memset(ones_cf, 1.0)
        ones_c = const.tile([P, 1], f32r)
        nc.vector.tensor_copy(ones_c, ones_cf)
        eps_sb = const.tile([1, 1], f32)
        nc.vector.memset(eps_sb, EPS)
        mr2ones = const.tile([2, TOK], bf16)
        nc.vector.memset(mr2ones, 1.0)  # row 0 overwritten with mu2*rstd2

        # collective bounce buffers
        cc1_in = dram.tile([DGR, TOK], fp8)
        cc1_out = dram.tile([NCORES, DGR, TOK], fp8, addr_space="Shared")
        cc2_in = [dram.tile([NCORES, DH + 1, TOK], bf16, name=f"c2i{h}")
                  for h in range(HL)]
        cc2_out = [dram.tile([NCORES, DH + 1, TOK], bf16, name=f"c2o{h}")
                   for h in range(HL)]

        # ---- phase 1: LN1 stats, xs = x*rstd (fp8), mr row -> AllGather ----
        s0 = ExitStack()
        psS = s0.enter_context(tc.tile_pool(name="psS", bufs=1, space="PSUM"))
        lnw = s0.enter_context(tc.tile_pool(name="lnw", bufs=2))
        xb = lnw.tile([P, KD, TOK], bf16, tag="xb", bufs=1)
        for k in range(KD):
            nc.sync.dma_start(xb[:, k, 0:TOK // 2], xv[:, k, 0:TOK // 2])
            nc.sync.dma_start(xb[:, k, TOK // 2:], xv[:, k, TOK // 2:])
        # const DMAs issue after the critical-path input chunks
        nc.sync.dma_start(awg_sb, awg)
        nc.sync.dma_start(cr_sb, cr)
        nc.sync.dma_start(mka, mk)
        nc.sync.dma_start(pb_sb, pb)
        nc.sync.dma_start(fcc_sb, fcc)
        nc.sync.dma_start(gb_sb, gb)
        with nc.named_scope("ln1"):
            sx1 = psS.tile([1, TOK], f32, tag="s1")
            sx2 = psS.tile([1, TOK], f32, tag="s2")
            for k in range(KD):
                sq = lnw.tile([P, TOK], bf16, tag="sq")
                nc.vector.tensor_mul(sq, xb[:, k, :], xb[:, k, :])
                nc.tensor.matmul(sx1, ones_cb, xb[:, k, :],
                                 start=(k == 0), stop=(k == KD - 1),
                                 skip_group_check=True)
                nc.tensor.matmul(sx2, ones_cb, sq,
                                 start=(k == 0), stop=(k == KD - 1),
                                 skip_group_check=True)
            mu = rows.tile([1, TOK], f32, tag="row")
            nc.vector.tensor_scalar_mul(mu, sx1, 1.0 / D)
            m2 = rows.tile([1, TOK], f32, tag="row")
            nc.vector.tensor_scalar_mul(m2, sx2, 1.0 / D)
            var = rows.tile([1, TOK], f32, tag="row")
            nc.vector.tensor_tensor(out=var, in0=mu, in1=mu, op=OP.mult)
            nc.vector.tensor_tensor(out=var, in0=m2, in1=var, op=OP.subtract)
            rstd = rows.tile([1, TOK], f32r, tag="row")
            act_raw(rstd, var, AF.Rsqrt, bias=eps_sb[:])
            mr = rows.tile([1, TOK], fp8, tag="mrow")
            nc.vector.tensor_tensor(out=mr, in0=mu, in1=rstd, op=OP.mult)
            nc.sync.dma_start(cc1_in[D:D + 1, :], mr)
            nc.sync.dma_start(cc1_in[D + 1:D + 2, :], ones_tok)
            rstd_b = psum.tile([P, TOK], f32, tag="big", bufs=2)
            nc.tensor.matmul(rstd_b, ones_r, rstd, start=True, stop=True)
            cc1v = cc1_in[0:D, :].rearrange("(k p) c -> p k c", p=P)
            rstd_s = lnw.tile([P, TOK], f32, tag="rs", bufs=1)
            nc.vector.tensor_copy(rstd_s, rstd_b)
            xs8 = lnw.tile([P, KD, TOK], fp8, tag="x8", bufs=1)
            for k in range(KD):
                eng = nc.vector if k % 2 == 0 else nc.gpsimd
                eng.tensor_tensor(out=xs8[:, k, :], in0=xb[:, k, :],
                                  in1=rstd_s, op=OP.mult)
                nc.sync.dma_start(cc1v[:, k, :], xs8[:, k, :])
        with nc.named_scope("ag"):
            nc.gpsimd.collective_compute(
                "AllGather", OP.bypass, replica_groups=RG,
                ins=[cc1_in[:].opt()], outs=[cc1_out[:].opt()])
        s0.close()  # free the LN1 stats PSUM banks for the attention pools

        att2 = ctx.enter_context(tc.tile_pool(name="att2", bufs=1))
        # attention-era pools (freed before the MLP pools allocate)
        s1 = ExitStack()
        ares = s1.enter_context(tc.tile_pool(name="ares", bufs=1))
        xnp = s1.enter_context(tc.tile_pool(name="xnp", bufs=1))
        wp = s1.enter_context(tc.tile_pool(name="wp", bufs=3))
        psA = s1.enter_context(tc.tile_pool(name="psA", bufs=2, space="PSUM"))

        # ---- phase 2: QKV (fp8 DoubleRow) for local heads, all tokens ----
        kT = [ares.tile([P, TOK], bf16, name=f"kT{r}") for r in range(NCORES)]
        qT = [ares.tile([P, TOK], bf16, name=f"qT{r}") for r in range(NCORES)]
        vA = [ares.tile([P, 4, HL, VP], fp8, name=f"vA{r}")
              for r in range(NCORES)]
        for r in range(NCORES):
            nc.vector.memset(vA[r][:, :, :, DH:DH + 1], 1.0)

        # gathered activations: 6-slot ring, prefetches staged so ranks 4-7
        # stream in under the attention exp shadow
        xs_t = {}
        mro_r = [xnp.tile([2, TOK], fp8, name=f"mro{r}")
                 for r in range(NCORES)]

        def prefetch(r):
            xs_t[r] = xnp.tile([P, KD, TOK], fp8, tag="xsr", bufs=5,
                               name=f"xsr{r}")
            # per-chunk DMAs spread across the 16 DMA engines (~22 GB/s each)
            cv = cc1_out[r, 0:D, :].rearrange("(k p) c -> p k c", p=P)
            for k in range(KD):
                nc.sync.dma_start(xs_t[r][:, k, :], cv[:, k, :])
            nc.sync.dma_start(mro_r[r], cc1_out[r, D:DGR, :])

        def emit_qk(r):
            for which in range(2):  # 0 -> q, 1 -> k
                ps = psum.tile([P, TOK], f32, tag="big", bufs=2)
                cb = which * P
                for kp in range(KD // 2):
                    nc.tensor.matmul(ps, awg_sb[:, 2 * kp:2 * kp + 2,
                                                cb:cb + P],
                                     xs_t[r][:, 2 * kp:2 * kp + 2, :],
                                     perf_mode=DR,
                                     start=(kp == 0), stop=False)
                nc.tensor.matmul(ps, cr_sb[:, which, :], mro_r[r],
                                 start=False, stop=True)
                dst = qT[r] if which == 0 else kT[r]
                nc.vector.tensor_scalar_mul(dst, ps, 1.0 / 16)

        def emit_v(r):
            for t in range(4):
                psv = psum.tile([P, P], f32, tag="big", bufs=2)
                for kp in range(KD // 2):
                    nc.tensor.matmul(psv,
                                     xs_t[r][:, 2 * kp:2 * kp + 2,
                                             P * t:P * (t + 1)],
                                     awg_sb[:, 2 * kp:2 * kp + 2, 2 * P:3 * P],
                                     perf_mode=DR,
                                     start=(kp == 0), stop=False)
                nc.tensor.matmul(psv, mro_r[r][:, P * t:P * (t + 1)],
                                 cr_sb[:, 2, :], start=False, stop=True)
                nc.vector.tensor_scalar_mul(
                    vA[r][:, t, :, 0:DH],
                    psv.rearrange("p (h d) -> p h d", h=HL), 1.0 / 16)

        # ---- phase 3: causal attention, head-major, software-pipelined ----
        aT = ares.tile([P, NCORES, TOK], bf16)
        drows = ares.tile([1, HL, NCORES, TOK], bf16)
        aFb = [att2.tile([P, TOK], bf16, name=f"aFb{r}")
               for r in range(NCORES)]
        aF8 = att2.tile([P, NCORES, TOK], fp8)
        d2 = [att2.tile([1, HL, TOK], bf16, name=f"d2{r}")
              for r in range(NCORES)]

        def ship(h, j):
            nc.sync.dma_start(cc2_in[h][j][0:DH], aT[DH * h:DH * h + DH, j, :])
            nc.sync.dma_start(cc2_in[h][j][DH:DH + 1],
                              drows[0:1, h, j, :])

        def fetch_back(h, r):
            nc.sync.dma_start(aFb[r][DH * h:DH * h + DH, :],
                              cc2_out[h][r][0:DH])
            nc.sync.dma_start(d2[r][0:1, h, :], cc2_out[h][r][DH:DH + 1])

        def attn_head(h, boundary_work):
            """boundary_work: dict (b, qc) -> [thunks] emitted after that
            query-chunk's last AV group is issued (PE filler under exp)."""
            hb = DH * h
            items = []
            for b in (0, 1):
                for qc in range(NQC):
                    nkb = 2 * qc + 2
                    kb0 = 0
                    while kb0 < nkb:
                        nblk = min(4, nkb - kb0)
                        items.append((b, qc, kb0, nblk))
                        kb0 += nblk
            accs = {}
            pending = None

            def emit_av(w8, it):
                b, qc, kb0, nblk = it
                nkb = 2 * qc + 2
                acc = accs[(b, qc)]
                for pr in range(nblk // 2):
                    kb = kb0 + 2 * pr
                    r = 4 * b + kb // 4
                    t = kb % 4
                    nc.tensor.matmul(acc, vA[r][:, t:t + 2, h, 0:DH + 1],
                                     w8[:, 2 * pr:2 * pr + 2, :],
                                     perf_mode=DR,
                                     start=(kb0 == 0 and pr == 0),
                                     stop=(kb + 2 == nkb),
                                     skip_group_check=True)
                if kb0 + nblk == nkb:  # last group of this query chunk
                    qr = 4 * b + qc // 2
                    qo = QCH * (qc % 2)
                    nc.vector.tensor_copy(aT[hb:hb + DH, qr, qo:qo + QCH],
                                          acc[0:DH, :])
                    nc.vector.tensor_copy(drows[0:1, h, qr, qo:qo + QCH],
                                          acc[DH:DH + 1, :])
                    del accs[(b, qc)]
                    if qc % 2 == 1:
                        ship(h, qr)
                    for thunk in boundary_work.get((b, qc), ()):
                        thunk()

            for it in items:
                b, qc, kb0, nblk = it
                qr = 4 * b + qc // 2
                qo = QCH * (qc % 2)
                if kb0 == 0:
                    accs[(b, qc)] = psA.tile([DH + 1, QCH], f32, tag="acc",
                                             bufs=2, name=f"acc{h}_{b}_{qc}")
                sc = psA.tile([P, 4, QCH], f32, tag="sc", bufs=2)
                for j in range(nblk):
                    kb = kb0 + j
                    r = 4 * b + kb // 4
                    t = kb % 4
                    nc.tensor.matmul(sc[:, j, :],
                                     kT[r][hb:hb + DH, P * t:P * (t + 1)],
                                     qT[qr][hb:hb + DH, qo:qo + QCH],
                                     start=True, stop=True,
                                     skip_group_check=True)
                if kb0 + nblk == 2 * qc + 2:  # diagonal pair: additive mask
                    dj = nblk - 2
                    nc.vector.tensor_tensor(out=sc[:, dj:dj + 2, :],
                                            in0=sc[:, dj:dj + 2, :],
                                            in1=mka, op=OP.add)
                w8 = wp.tile([P, 4, QCH], fp8, tag="w")
                nc.scalar.activation(w8[:, 0:nblk, :], sc[:, 0:nblk, :],
                                     AF.Exp, scale=0.125)
                if pending is not None:
                    emit_av(*pending)
                pending = (w8, it)
            emit_av(*pending)

        with nc.named_scope("qkv"):
            for r in range(4):
                prefetch(r)
            emit_qk(0)
            emit_v(0)

        # PE filler schedule: rank r's qk/v must exist before the first
        # query chunk that reads k-block 4*(r%4) of batch r//4. Prefetches
        # for ranks 4-7 stream in a few chunks ahead (6-slot ring).
        bw0 = {
            (0, 0): [lambda: emit_qk(1)],
            (0, 1): [lambda: emit_v(1), lambda: emit_qk(2),
                     lambda: prefetch(4)],
            (0, 2): [lambda: emit_v(2), lambda: prefetch(5)],
            (0, 3): [lambda: emit_qk(3), lambda: emit_v(3),
                     lambda: prefetch(6)],
            (0, 4): [lambda: emit_qk(4), lambda: emit_v(4),
                     lambda: prefetch(7)],
            (0, 5): [lambda: emit_qk(5), lambda: emit_v(5)],
            (0, 6): [lambda: emit_qk(6), lambda: emit_v(6)],
            (0, 7): [lambda: emit_qk(7), lambda: emit_v(7)],
        }
        with nc.named_scope("attn0"):
            attn_head(0, bw0)
        with nc.named_scope("a2a0"):
            nc.gpsimd.collective_compute(
                "AllToAll", OP.bypass, replica_groups=RG,
                ins=[cc2_in[0][:].opt()], outs=[cc2_out[0][:].opt()])
        # head 1: interleave head-0 result prefetches under the exp shadow
        bw1 = {(0, qc): [lambda r=qc: fetch_back(0, r)] for qc in range(NQC)}
        with nc.named_scope("attn1"):
            attn_head(1, bw1)
        with nc.named_scope("a2a1"):
            nc.gpsimd.collective_compute(
                "AllToAll", OP.bypass, replica_groups=RG,
                ins=[cc2_in[1][:].opt()], outs=[cc2_out[1][:].opt()])
        for r in range(NCORES):
            fetch_back(1, r)

        s1.close()  # release attention-era SBUF/PSUM
        s2 = ExitStack()
        psB = s2.enter_context(tc.tile_pool(name="psB", bufs=2, space="PSUM"))
        mlp = ctx.enter_context(tc.tile_pool(name="mlp", bufs=1))
        wgt = ctx.enter_context(tc.tile_pool(name="wgt", bufs=1))
        outp = ctx.enter_context(tc.tile_pool(name="outp", bufs=1))

        # ---- phase 4: normalize by softmax denominators + output proj ----
        h1T = mlp.tile([P, KD, TOK], f32)
        xbr = mlp.tile([P, KD, TOK], bf16)
        for k in range(KD):
            nc.sync.dma_start(xbr[:, k, :], xv[:, k, :])
        st2b = psB.tile([1, TOK], f32, tag="s1", bufs=1)
        st2c = psB.tile([1, TOK], f32, tag="s2", bufs=1)
        with nc.named_scope("proj"):
            # head-0 halves normalize while the head-1 AllToAll is still in
            # flight (their data arrived during attn1); head-1 after.
            # di carries a x32 scale so the fp8 aF8 uses its dynamic range
            # (descale folded into the proj drain).
            for h in range(HL):
                for r in range(NCORES):
                    di = outp.tile([1, TOK], bf16, tag="di", bufs=2)
                    act_raw(di, d2[r][0:1, h, :], AF.Reciprocal,
                            scale=1.0 / 32)
                    dbc = psB.tile([P, TOK], f32, tag="bc", bufs=2)
                    nc.tensor.matmul(dbc[DH * h:DH * h + DH, :],
                                     ones_tok_b[:, 0:DH], di,
                                     start=True, stop=True,
                                     skip_group_check=True)
                    nc.vector.tensor_tensor(
                        out=aF8[DH * h:DH * h + DH, r, :],
                        in0=aFb[r][DH * h:DH * h + DH, :],
                        in1=dbc[DH * h:DH * h + DH, :], op=OP.mult)
            for f in range(KD):
                pwt = wgt.tile([P, KD, P], fp8, tag="pw", bufs=2)
                nc.sync.dma_start(pwt, pw[f])
                ps = psum.tile([P, TOK], f32, tag="big", bufs=2)
                for k in range(KD // 2):
                    nc.tensor.matmul(ps, pwt[:, 2 * k:2 * k + 2, :],
                                     aF8[:, 2 * k:2 * k + 2, :],
                                     perf_mode=DR,
                                     start=(k == 0), stop=(k == KD // 2 - 1))
                t1 = lnt.tile([P, TOK], f32, tag="pj")
                nc.vector.tensor_scalar(out=t1, in0=ps,
                                        scalar1=1.0 / 512,
                                        scalar2=pb_sb[:, f:f + 1],
                                        op0=OP.mult, op1=OP.add)
                nc.vector.tensor_tensor(out=h1T[:, f, :], in0=t1,
                                        in1=xbr[:, f, :], op=OP.add)
                xr = sqp.tile([P, TOK], f32r, tag="xr")
                nc.vector.tensor_copy(xr, h1T[:, f, :])
                sq = sqp.tile([P, TOK], f32r, tag="sq2")
                nc.vector.tensor_mul(sq, h1T[:, f, :], h1T[:, f, :])
                nc.tensor.matmul(st2b, ones_c, xr,
                                 start=(f == 0), stop=(f == KD - 1),
                                 skip_group_check=True)
                nc.tensor.matmul(st2c, ones_c, sq,
                                 start=(f == 0), stop=(f == KD - 1),
                                 skip_group_check=True)

        # ---- phase 5: LN2 (folded into fc1) -> xs2 = h1*rstd2 (bf16) ----
        xs2 = mlp.tile([P, KD, TOK], bf16)
        with nc.named_scope("ln2"):
            mu2 = rows.tile([1, TOK], f32, tag="row")
            nc.vector.tensor_scalar_mul(mu2, st2b, 1.0 / D)
            m22 = rows.tile([1, TOK], f32, tag="row")
            nc.vector.tensor_scalar_mul(m22, st2c, 1.0 / D)
            var2 = rows.tile([1, TOK], f32, tag="row")
            nc.vector.tensor_tensor(out=var2, in0=mu2, in1=mu2, op=OP.mult)
            nc.vector.tensor_tensor(out=var2, in0=m22, in1=var2,
                                    op=OP.subtract)
            rstd2 = rows.tile([1, TOK], f32r, tag="row")
            act_raw(rstd2, var2, AF.Rsqrt, bias=eps_sb[:])
            nc.vector.tensor_tensor(out=mr2ones[0:1, :], in0=mu2, in1=rstd2,
                                    op=OP.mult)
            rsb = psB.tile([P, TOK], f32, tag="bc", bufs=2)
            nc.tensor.matmul(rsb, ones_r, rstd2, start=True, stop=True)
            rsb_s = lnt.tile([P, TOK], f32, tag="pj")
            nc.vector.tensor_copy(rsb_s, rsb)
            for k in range(KD):
                eng = nc.vector if k % 2 == 0 else nc.gpsimd
                eng.tensor_tensor(out=xs2[:, k, :], in0=h1T[:, k, :],
                                  in1=rsb_s, op=OP.mult)

        s2.close()  # free proj/LN2 PSUM before the fc1 pool opens
        s3 = ExitStack()
        psF = s3.enter_context(tc.tile_pool(name="psF", bufs=2, space="PSUM"))

        # ---- phase 6: MLP (LN2-folded fc1 + batched gelu, then fc2) ----
        hT = mlp.tile([P, KDI, TOK], bf16)
        # hoist the first weight tile's DMA so it streams in during ln2
        def fw_tile(jp, half):
            t = wgt.tile([P, KD, P], bf16, tag="fw", bufs=3,
                         name=f"fwt{jp}_{half}")
            nc.sync.dma_start(t, fw[jp, half])
            return t
        fwt0 = fw_tile(0, 0)
        with nc.named_scope("fc1"):
            for jp in range(KDI // 2):
                ps = psF.tile([P, 2, TOK], f32, tag="f1", bufs=2)
                for half in range(2):
                    j = 2 * jp + half
                    fwt = fwt0 if j == 0 else fw_tile(jp, half)
                    for k in range(KD):
                        nc.tensor.matmul(ps[:, half, :], fwt[:, k, :],
                                         xs2[:, k, :],
                                         start=(k == 0), stop=False,
                                         skip_group_check=True)
                    nc.tensor.matmul(ps[:, half, :], fcc_sb[:, j, :],
                                     mr2ones, start=False, stop=True,
                                     skip_group_check=True)
                nc.scalar.activation(hT[:, 2 * jp:2 * jp + 2, :], ps,
                                     AF.Gelu_apprx_tanh)
        with nc.named_scope("fc2"):
            KH = KDI // 4
            for f in range(KD):
                ps = psum.tile([P, TOK], f32, tag="big", bufs=2)
                for half in range(4):
                    gwt = wgt.tile([P, KH, P], bf16, tag="gw", bufs=3)
                    nc.sync.dma_start(gwt, gw[f, half])
                    for kk in range(KH):
                        k = KH * half + kk
                        nc.tensor.matmul(ps, gwt[:, kk, :], hT[:, k, :],
                                         start=(k == 0),
                                         stop=(k == KDI - 1))
                o = outp.tile([P, TOK], f32, tag="ot", bufs=2)
                nc.vector.tensor_scalar_add(o, ps, gb_sb[:, f:f + 1])
                nc.vector.tensor_tensor(out=o, in0=o, in1=h1T[:, f, :],
                                        op=OP.add)
                nc.sync.dma_start(outT[P * f:P * (f + 1), :], o)
        s3.close()

    nc.compile()
    return nc


def shard_inputs(inputs):
    """Full inputs -> list of 8 per-core input dicts (host-side layout only)."""
    bf16 = ml_dtypes.bfloat16
    f32 = np.float32
    f8 = ml_dtypes.float8_e4m3fn
    hs = np.asarray(inputs["hidden_states"], f32).reshape(TT, D)
    attn_w = np.asarray(inputs["attn_w"], f32)
    attn_b = np.asarray(inputs["attn_b"], f32)
    l1g = np.asarray(inputs["ln1_g"], f32)
    l1b = np.asarray(inputs["ln1_b"], f32)
    l2g = np.asarray(inputs["ln2_g"], f32)
    l2b = np.asarray(inputs["ln2_b"], f32)

    def col(v):  # [D] -> [P, KD]
        return np.ascontiguousarray(np.asarray(v, f32).reshape(KD, P).T)

    pw = np.ascontiguousarray((np.asarray(inputs["proj_w"], f32) * 16)
                              .reshape(KD, P, KD, P).transpose(2, 1, 0, 3)
                              .astype(f8))
    # fc1: fold LN2 gamma into the weights; beta@W + fc_b and the colsum
    # correction go into the 2-row fcc tensor. Pair layout for batched gelu.
    fw_full = np.asarray(inputs["fc_w"], f32)
    fwg = fw_full * l2g[:, None]
    c1 = fwg.sum(axis=0)                              # [DI]
    fbe = l2b @ fw_full + np.asarray(inputs["fc_b"], f32)
    fcc = np.ascontiguousarray(
        np.stack([-c1, fbe]).reshape(2, KDI, P).astype(bf16))
    fw = np.ascontiguousarray(
        fwg.reshape(KD, P, KDI // 2, 2, P).transpose(2, 3, 1, 0, 4)
        .astype(bf16))
    gw = np.ascontiguousarray(np.asarray(inputs["fc2_w"], f32)
                              .reshape(4, KDI // 4, P, KD, P)
                              .transpose(3, 0, 2, 1, 4)
                              .astype(bf16))
    pb = col(inputs["proj_b"])
    gbv = col(inputs["fc2_b"])

    # additive causal masks for the diagonal k-block pair, pre-exp on PSUM
    # (exp applies scale 0.125 to score+mask, so -1e5 -> exp -> 0)
    ii, jj = np.meshgrid(np.arange(P), np.arange(QCH), indexing="ij")
    mk = np.stack([np.where(jj >= ii, 0.0, -1e5),
                   np.where(jj >= ii + P, 0.0, -1e5)], axis=1).astype(bf16)
    mk = np.ascontiguousarray(mk)                     # [P, 2, QCH]

    maps = []
    for c in range(NCORES):
        cols = np.r_[P * c:P * (c + 1),
                     D + P * c:D + P * (c + 1),
                     2 * D + P * c:2 * D + P * (c + 1)]
        w_c = attn_w[:, cols]                      # [D, 384]
        wg_c = w_c * l1g[:, None]                  # gamma folded
        c1a = wg_c.sum(axis=0)                     # [384]
        cba = l1b @ w_c + attn_b[cols]             # [384]
        cr_c = np.ascontiguousarray((np.stack([
            np.stack([-c1a[0:P], -c1a[P:2 * P], -c1a[2 * P:3 * P]]),
            np.stack([cba[0:P], cba[P:2 * P], cba[2 * P:3 * P]]),
        ]) * 16).astype(f8))
        awg_c = np.ascontiguousarray(
            (wg_c * 16).reshape(KD, P, 3 * P).transpose(1, 0, 2).astype(f8))
        xTb_c = np.ascontiguousarray(hs[TOK * c:TOK * (c + 1)].T.astype(bf16))
        maps.append({
            "xTb": xTb_c, "awg": awg_c, "cr": cr_c,
            "pw": pw, "pb": pb, "fw": fw, "fcc": fcc, "gw": gw, "gb": gbv,
            "mk": mk,
        })
    return maps


def unshard(results):
    out = np.concatenate([np.asarray(r["outT"]).T for r in results], axis=0)
    return np.ascontiguousarray(out.reshape(B, S, D))


def kernel(**inputs):
    global _CACHED_NC
    from concourse.bass_utils import run_bass_kernel_spmd
    if _CACHED_NC is None:
        _CACHED_NC = build_nc()
    in_maps = shard_inputs(inputs)
    res = run_bass_kernel_spmd(_CACHED_NC, in_maps,
                               core_ids=list(range(NCORES)))
    return unshard(res.results)
